# revision 6
# baseline (speedup 1.0000x reference)
"""Batched int8 GEMM (s8t x s8n -> s32t) on 8 TRN2 NeuronCores.

out[b, m, n] = sum_k a[b, m, k] * b[b, n, k]   (int32 accumulation)
a: [32, 1024, 1024] int8, b: [32, 1024, 1024] int8 -> out: [32, 1024, 1024] int32

Strategy:
  - Pure batch parallelism: 4 batches per core across 8 cores.
  - Both operands have K innermost, but the PE needs K on partitions.
    DMA-transpose works on 2-byte elements only, so we view the int8
    inputs as uint16 (pairs of adjacent K values) and DMA-transpose
    K-blocks of 256 K-values; each partition holds an even/odd K pair
    interleaved along the free dim. DVE deinterleaves (stride-2 int8
    reads) and converts int8 -> bf16: int8 is exact in bf16; products
    <= 2^14 and sums <= 2^24 are exact in fp32 PSUM accumulation, so
    the GEMM is bit-exact.
  - ALL transposes on the SYNC HWDGE queue: concurrent DMA-transposes
    on the sync+scalar queues race on the shared xbar and corrupt the
    staged tiles (measured). DMA_TRANSPOSE issue cost is ~1.3us FLAT
    regardless of size, so k-blocks transpose as FULL [1024,128]u16
    tiles (halves only where latency matters: batch 0 k-block 0, and
    A's upper half which is only needed at mt group 1).
  - Batch 0 issue order B-h0, A-h0, B-h1, then kb1-3 A+B fulls: the
    first matmul gates on B-h0+A-h0 (~1us earlier than A-first), and
    each arriving k-block feeds 1.7us of PE work (kt-outer over two
    groups of 4 mt blocks = 8 PSUM banks). Deints for B are emitted
    before A within each kt for the same reason. 9 dep-free dummy
    matmuls warm the HAM clock gate until the first real matmul.
  - Batch 1 + batch 2's first k-block transposes issue up front; the
    rest thread through batch 0/1's store stream on the sync FIFO
    (store, transpose, store, ... keeps every DMA's completion-
    semaphore-lane predecessor recent AND keeps stores flowing so the
    8-deep output-buffer ring never backs up into the ACT copies).
  - ACT copies PSUM fp32 -> SBUF int32 (exact: values are integers).
    Batches 0/1 stores issue from SYNC (interleaved with the pending
    transposes, as above). Batches 2/3 split: even mt rows on SYNC,
    odd on SCALAR - by then the transpose stream has fully drained,
    so the two store queues run in parallel and the tail drains 2x
    faster (single-queue stores previously backed up ~3us at the
    end).
  - The final mt row runs nt-outer (kt inner per nt) so nt0's copy +
    256KB store overlap nt1's matmuls; nt1 finishes with parallel
    ACT+DVE half copies and two 128KB stores on both queues, so the
    kernel tail waits only on the last 128KB.
"""

import numpy as np

import concourse.mybir as mybir
import concourse.tile as tile
from concourse import bacc
from concourse.bass_utils import run_bass_kernel_spmd

B, M, N, K = 32, 1024, 1024, 1024
N_CORES = 8
BPC = B // N_CORES  # batches per core
KB = K // 256  # k-blocks of 256 K-values (128 uint16 partitions)
N_TILE = 512
M_TILE = 128
WARMUP_MMS = 9

_nc_cache = None


def build_nc():
    nc = bacc.Bacc("TRN2")

    # int8 inputs viewed as uint16 so the xbar DMA-transpose (2-byte
    # granularity) can be used straight out of HBM.
    a_in = nc.dram_tensor("a", [BPC, M, K // 2], mybir.dt.uint16, kind="ExternalInput")
    b_in = nc.dram_tensor("b", [BPC, N, K // 2], mybir.dt.uint16, kind="ExternalInput")
    out = nc.dram_tensor("out", [BPC, M, N], mybir.dt.int32, kind="ExternalOutput")

    n_mt = M // M_TILE
    n_nt = N // N_TILE
    n_kt = 2 * KB

    with tile.TileContext(nc) as tc:
        with (
            tc.tile_pool(name="stage", bufs=1) as stage_pool,
            tc.tile_pool(name="conv", bufs=2) as conv_pool,
            tc.tile_pool(name="psum", bufs=8, space="PSUM") as psum_pool,
            tc.tile_pool(name="outbuf", bufs=10) as out_pool,
            tc.tile_pool(name="warm", bufs=1) as warm_pool,
        ):
            # PE warmup: dummy matmuls with NO deps (uninitialized SBUF
            # reads are fine; the PSUM result is discarded) ramp the HAM
            # clock gate while the first transposes land.
            wsrc = warm_pool.tile([128, N_TILE], mybir.dt.bfloat16, name="wsrc")
            nc.vector.memset(wsrc[:, :8], 0.0)
            wps = psum_pool.tile([128, N_TILE], mybir.dt.float32, name="wps", tag="ps")
            for _ in range(WARMUP_MMS):
                nc.tensor.matmul(wps[:], wsrc[:, :128], wsrc[:], start=True, stop=True)

            # ---- all transposes up front on SYNC. Batch 0: k-block 0 in
            # halves ordered B-h0, A-h0, B-h1 (the first matmuls need
            # B-h0 + A-h0), kb1-3 full tiles, A-h1 last. ----
            def stage_small(src, kb, name, m0, m1):
                t = stage_pool.tile(
                    [128, m1 - m0],
                    mybir.dt.uint16,
                    name=f"{name}0_{kb}_{m0}",
                    tag=f"{name}0_{kb}_{m0}",
                )
                nc.sync.dma_start_transpose(t[:], src[0, m0:m1, kb * 128 : (kb + 1) * 128])
                # (int8 view, m0, m1, byte offset of m0 within the tile)
                return (t.bitcast(mybir.dt.int8), m0, m1, 0)

            def stage_small_b(src, bi, kb, name):
                t = stage_pool.tile(
                    [128, M],
                    mybir.dt.uint16,
                    name=f"{name}{bi}_{kb}",
                    tag=f"{name}{bi}_{kb}",
                )
                nc.sync.dma_start_transpose(
                    t[:], src[bi, :, kb * 128 : (kb + 1) * 128]
                )
                return (t.bitcast(mybir.dt.int8), 0, M, 0)

            a_sm = {0: []}
            b_sm = {0: []}
            b_sm[0].append(stage_small(b_in, 0, "bt", 0, M // 2))
            a_sm[0].append(stage_small(a_in, 0, "at", 0, M // 2))
            b_sm[0].append(stage_small(b_in, 0, "bt", M // 2, M))
            for kb in range(1, KB):
                a_sm[kb] = [stage_small(a_in, kb, "at", 0, M)]
                b_sm[kb] = [stage_small(b_in, kb, "bt", 0, M)]
            a_sm[0].append(stage_small(a_in, 0, "at", M // 2, M))

            # Batches 1-3: batch 1 + batch 2's k-block 0 up front; the
            # rest thread through the store stream (each sync store pops
            # one pending transpose).
            a_smb = {}
            b_smb = {}

            def stage_one(kind, bi, kb):
                if kind == "A":
                    a_smb.setdefault(bi, {})[kb] = [stage_small_b(a_in, bi, kb, "at")]
                else:
                    b_smb.setdefault(bi, {})[kb] = [stage_small_b(b_in, bi, kb, "bt")]

            for kb in range(KB):
                stage_one("A", 1, kb)
                stage_one("B", 1, kb)
            stage_one("A", 2, 0)
            stage_one("B", 2, 0)
            pending_by_batch = {
                0: [(k, 2, kb) for kb in (1, 2, 3) for k in ("A", "B")],
                1: [(k, 3, kb) for kb in range(KB) for k in ("A", "B")],
            }
            pending_T = []

            for bi in range(BPC):
                pending_T = pending_by_batch.get(bi, [])
                # ---- deinterleave + int8 -> bf16 (DVE). lhs_ap[kt][mt]
                # and rhs_ap[kt][nt] index into whichever tile holds that
                # m/n range. ----
                lhs_ap = [[None] * n_mt for _ in range(n_kt)]
                rhs_ap = [[None] * n_nt for _ in range(n_kt)]
                deferred_a = []  # batch 0's (kb, A-h1 seg): deints go last
                for kb in range(KB):
                    if bi == 0:
                        # Batch 0 deints in HALVES (sliced from the full
                        # staged tiles for kb>=1): mt group 0 never reads
                        # A's upper half (deints deferred past kb3), and
                        # B's h0 unlocks each kt's nt0 matmuls sooner.
                        if kb == 0:
                            a_segs = a_sm[kb]
                            b_segs = b_sm[kb]
                        else:
                            at8 = a_sm[kb][0][0]
                            bt8 = b_sm[kb][0][0]
                            a_segs = [(at8, 0, M // 2, 0), (at8, M // 2, M, M)]
                            b_segs = [(bt8, 0, M // 2, 0), (bt8, M // 2, M, M)]
                        deferred_a += [(kb, s) for s in a_segs if s[1] != 0]
                        a_segs = [s for s in a_segs if s[1] == 0]
                    else:
                        a_segs = a_smb[bi][kb]
                        b_segs = b_smb[bi][kb]
                    for par in range(2):
                        kt = kb * 2 + par
                        # B deints first: each kt's first matmuls gate on
                        # B-h0 + A, and B's transpose is issued first.
                        for st8, m0, m1, base in b_segs:
                            bbf = conv_pool.tile(
                                [128, m1 - m0],
                                mybir.dt.bfloat16,
                                name=f"bbf_{bi}_{kt}_{m0}",
                                tag=f"bbf{kt}_{m0}",
                            )
                            # All deints on DVE: ACT's ACTIVATE-copy runs
                            # int8 deints ~1.65x slower.
                            nc.vector.tensor_copy(
                                bbf[:],
                                st8[:, base + par : base + 2 * (m1 - m0) : 2],
                            )
                            for nt in range(m0 // N_TILE, (m1 + N_TILE - 1) // N_TILE):
                                o = nt * N_TILE - m0
                                rhs_ap[kt][nt] = bbf[:, o : o + N_TILE]
                        for st8, m0, m1, base in a_segs:
                            abf = conv_pool.tile(
                                [128, m1 - m0],
                                mybir.dt.bfloat16,
                                name=f"abf_{bi}_{kt}_{m0}",
                                tag=f"abf{kt}_{m0}",
                            )
                            nc.vector.tensor_copy(
                                abf[:],
                                st8[:, base + par : base + 2 * (m1 - m0) : 2],
                            )
                            for mt in range(m0 // M_TILE, m1 // M_TILE):
                                o = mt * M_TILE - m0
                                lhs_ap[kt][mt] = abf[:, o : o + M_TILE]

                # Batch 0's A-h1 deints last on the DVE FIFO: that data
                # (lhs for mt 4-7) is only needed at mt group 1 (~14us
                # after the ramp starts), and its transposes issue late.
                for kb, (st8, m0, m1, base) in deferred_a:
                    for par in range(2):
                        kt = kb * 2 + par
                        abf = conv_pool.tile(
                            [128, m1 - m0],
                            mybir.dt.bfloat16,
                            name=f"abf_{bi}_{kt}_{m0}",
                            tag=f"abf{kt}_{m0}",
                        )
                        nc.vector.tensor_copy(
                            abf[:],
                            st8[:, base + par : base + 2 * (m1 - m0) : 2],
                        )
                        for mt in range(m0 // M_TILE, m1 // M_TILE):
                            o = mt * M_TILE - m0
                            lhs_ap[kt][mt] = abf[:, o : o + M_TILE]

                # ---- GEMM, accumulating in PSUM over kt. After each mt
                # row's two PSUM-freeing copies (ACT), the row's 512KB
                # store issues: even mt rows on SYNC, odd on SCALAR. A
                # single store queue is oversubscribed mid-kernel
                # (transposes 8MB + stores 16MB ~= 268GB/s demand vs
                # ~200GB/s per queue - measured: store backlog filled
                # the 8-deep output ring, stalled the ACT copies and
                # starved the PE ~4.6us on jittery runs). Each sync
                # store pops one pending transpose (keeps the sync FIFO
                # interleaved T,S,T,S as before). ----
                def emit_store(mt, ot):
                    if mt % 2 == 1:
                        nc.scalar.dma_start(
                            out[bi, mt * M_TILE : (mt + 1) * M_TILE, :], ot[:]
                        )
                    else:
                        nc.sync.dma_start(
                            out[bi, mt * M_TILE : (mt + 1) * M_TILE, :], ot[:]
                        )
                    if pending_T:
                        stage_one(*pending_T.pop(0))

                if bi == 0:
                    # Batch 0 is rate-limited by the transpose + deint
                    # stream: iterate kt-outer over groups of 4 mt blocks
                    # (8 PSUM banks) so each arriving k-tile feeds 1.7us
                    # of real PE work. nt-outer inside each kt so the
                    # first 4 matmuls only need B-half0.
                    for g in range(n_mt // 4):
                        mts = range(4 * g, 4 * g + 4)
                        ps = {
                            (mt, nt): psum_pool.tile(
                                [128, N_TILE],
                                mybir.dt.float32,
                                name=f"ps_{bi}_{mt}_{nt}",
                                tag="ps",
                            )
                            for mt in mts
                            for nt in range(n_nt)
                        }
                        for kt in range(n_kt):
                            for nt in range(n_nt):
                                for mt in mts:
                                    nc.tensor.matmul(
                                        ps[(mt, nt)][:],
                                        lhs_ap[kt][mt],
                                        rhs_ap[kt][nt],
                                        start=(kt == 0),
                                        stop=(kt == n_kt - 1),
                                    )
                        for mt in mts:
                            ot = out_pool.tile(
                                [128, N], mybir.dt.int32, name=f"ot_{bi}_{mt}", tag="ot"
                            )
                            for nt in range(n_nt):
                                nc.scalar.copy(
                                    ot[:, nt * N_TILE : (nt + 1) * N_TILE],
                                    ps[(mt, nt)][:],
                                )
                            emit_store(mt, ot)
                elif bi == BPC - 1:
                    # Last batch: mt-outer, but the FINAL row runs
                    # nt-outer so nt0's copy+store overlap nt1's matmuls
                    # and the kernel tail waits only on the last 128KB.
                    for mt in range(n_mt - 1):
                        ps = [
                            psum_pool.tile(
                                [128, N_TILE],
                                mybir.dt.float32,
                                name=f"ps_{bi}_{mt}_{nt}",
                                tag="ps",
                            )
                            for nt in range(n_nt)
                        ]
                        for kt in range(n_kt):
                            for nt in range(n_nt):
                                nc.tensor.matmul(
                                    ps[nt][:],
                                    lhs_ap[kt][mt],
                                    rhs_ap[kt][nt],
                                    start=(kt == 0),
                                    stop=(kt == n_kt - 1),
                                )
                        ot = out_pool.tile(
                            [128, N], mybir.dt.int32, name=f"ot_{bi}_{mt}", tag="ot"
                        )
                        for nt in range(n_nt):
                            nc.scalar.copy(
                                ot[:, nt * N_TILE : (nt + 1) * N_TILE], ps[nt][:]
                            )
                        emit_store(mt, ot)

                    mt = n_mt - 1
                    ps = [
                        psum_pool.tile(
                            [128, N_TILE],
                            mybir.dt.float32,
                            name=f"ps_{bi}_{mt}_{nt}",
                            tag="ps",
                        )
                        for nt in range(n_nt)
                    ]
                    # nt0 chain first, then nt1: nt0's copy + 256KB store
                    # run during nt1's 8 matmuls.
                    for nt in range(n_nt):
                        for kt in range(n_kt):
                            nc.tensor.matmul(
                                ps[nt][:],
                                lhs_ap[kt][mt],
                                rhs_ap[kt][nt],
                                start=(kt == 0),
                                stop=(kt == n_kt - 1),
                            )
                        if nt == 0:
                            o0 = out_pool.tile(
                                [128, N_TILE],
                                mybir.dt.int32,
                                name="ot_l0",
                                tag="otl0",
                                bufs=1,
                            )
                            nc.scalar.copy(o0[:], ps[0][:])
                            nc.scalar.dma_start(
                                out[bi, mt * M_TILE : (mt + 1) * M_TILE, :N_TILE],
                                o0[:],
                            )
                    # nt1: parallel ACT+DVE half copies, two 128KB stores
                    # on both queues; the tail waits only on the last one.
                    o1 = out_pool.tile(
                        [128, N_TILE],
                        mybir.dt.int32,
                        name="ot_l1",
                        tag="otl1",
                        bufs=1,
                    )
                    h = N_TILE // 2
                    nc.scalar.copy(o1[:, :h], ps[1][:, :h])
                    nc.vector.tensor_copy(o1[:, h:], ps[1][:, h:])
                    nc.scalar.dma_start(
                        out[bi, mt * M_TILE : (mt + 1) * M_TILE, N_TILE : N_TILE + h],
                        o1[:, :h],
                    )
                    nc.sync.dma_start(
                        out[bi, mt * M_TILE : (mt + 1) * M_TILE, N_TILE + h :],
                        o1[:, h:],
                    )
                else:
                    # Steady-state batches: mt-outer so the PSUM-freeing
                    # copies and stores spread evenly.
                    for mt in range(n_mt):
                        ps = [
                            psum_pool.tile(
                                [128, N_TILE],
                                mybir.dt.float32,
                                name=f"ps_{bi}_{mt}_{nt}",
                                tag="ps",
                            )
                            for nt in range(n_nt)
                        ]
                        for kt in range(n_kt):
                            for nt in range(n_nt):
                                nc.tensor.matmul(
                                    ps[nt][:],
                                    lhs_ap[kt][mt],
                                    rhs_ap[kt][nt],
                                    start=(kt == 0),
                                    stop=(kt == n_kt - 1),
                                )
                        ot = out_pool.tile(
                            [128, N], mybir.dt.int32, name=f"ot_{bi}_{mt}", tag="ot"
                        )
                        for nt in range(n_nt):
                            nc.scalar.copy(
                                ot[:, nt * N_TILE : (nt + 1) * N_TILE], ps[nt][:]
                            )
                        emit_store(mt, ot)
    nc.compile()
    return nc


def _get_nc():
    global _nc_cache
    if _nc_cache is None:
        _nc_cache = build_nc()
    return _nc_cache


def run(a: np.ndarray, b: np.ndarray, trace: bool = False):
    """Run on 8 cores. a/b: [32, 1024, 1024] int8. Returns (out, BassKernelResults)."""
    a = np.ascontiguousarray(a)
    b = np.ascontiguousarray(b)
    a16 = a.view(np.uint16).reshape(B, M, K // 2)
    b16 = b.view(np.uint16).reshape(B, N, K // 2)
    in_maps = [
        {
            "a": a16[c * BPC : (c + 1) * BPC],
            "b": b16[c * BPC : (c + 1) * BPC],
        }
        for c in range(N_CORES)
    ]
    res = run_bass_kernel_spmd(_get_nc(), in_maps, list(range(N_CORES)), trace=trace)
    out = np.concatenate([res.results[c]["out"] for c in range(N_CORES)], axis=0)
    return out, res


def kernel(a: np.ndarray, b: np.ndarray) -> np.ndarray:
    out, _ = run(np.asarray(a), np.asarray(b))
    return out


# revision 10
# speedup vs baseline: 1.0829x; 1.0829x over previous
"""Batched int8 GEMM (s8t x s8n -> s32t) on 8 TRN2 NeuronCores.

out[b, m, n] = sum_k a[b, m, k] * b[b, n, k]   (int32 accumulation)
a: [32, 1024, 1024] int8, b: [32, 1024, 1024] int8 -> out: [32, 1024, 1024] int32

Strategy:
  - Pure batch parallelism: 4 batches per core across 8 cores.
  - Both operands have K innermost, but the PE needs K on partitions.
    DMA-transpose works on 2-byte elements only, so we view the int8
    inputs as uint16 (pairs of adjacent K values) and DMA-transpose
    K-blocks of 256 K-values; each partition holds an even/odd K pair
    interleaved along the free dim. DVE deinterleaves (stride-2 int8
    reads) and converts int8 -> bf16: int8 is exact in bf16; products
    <= 2^14 and sums <= 2^24 are exact in fp32 PSUM accumulation, so
    the GEMM is bit-exact.
  - ALL transposes on the SYNC HWDGE queue: concurrent DMA-transposes
    on the sync+scalar queues race on the shared xbar and corrupt the
    staged tiles (measured). DMA_TRANSPOSE issue cost is ~1.3us FLAT
    regardless of size, so k-blocks transpose as FULL [1024,128]u16
    tiles (halves only where latency matters: batch 0 k-block 0, and
    A's upper half which is only needed at mt group 1).
  - Batch 0 issue order B-h0, A-h0, B-h1, then kb1-3 A+B fulls: the
    first matmul gates on B-h0+A-h0 (~1us earlier than A-first), and
    each arriving k-block feeds 1.7us of PE work (kt-outer over two
    groups of 4 mt blocks = 8 PSUM banks). Deints for B are emitted
    before A within each kt for the same reason. 9 dep-free dummy
    matmuls warm the HAM clock gate until the first real matmul.
  - Batch 1 + batch 2's first k-block transposes issue up front; the
    rest thread through batch 0/1's store stream on the sync FIFO
    (store, transpose, store, ... keeps every DMA's completion-
    semaphore-lane predecessor recent AND keeps stores flowing so the
    8-deep output-buffer ring never backs up into the ACT copies).
  - ACT copies PSUM fp32 -> SBUF int32 (exact: values are integers).
    Batches 0/1 stores issue from SYNC (interleaved with the pending
    transposes, as above). Batches 2/3 split: even mt rows on SYNC,
    odd on SCALAR - by then the transpose stream has fully drained,
    so the two store queues run in parallel and the tail drains 2x
    faster (single-queue stores previously backed up ~3us at the
    end).
  - The final mt row runs nt-outer (kt inner per nt) so nt0's copy +
    256KB store overlap nt1's matmuls; nt1 finishes with parallel
    ACT+DVE half copies and two 128KB stores on both queues, so the
    kernel tail waits only on the last 128KB.
"""

import numpy as np

import concourse.mybir as mybir
import concourse.tile as tile
from concourse import bacc
from concourse.bass_utils import run_bass_kernel_spmd

B, M, N, K = 32, 1024, 1024, 1024
N_CORES = 8
BPC = B // N_CORES  # batches per core
KB = K // 256  # k-blocks of 256 K-values (128 uint16 partitions)
N_TILE = 512
M_TILE = 128
WARMUP_MMS = 9

_nc_cache = None


def build_nc():
    nc = bacc.Bacc("TRN2")

    # int8 inputs viewed as uint16 so the xbar DMA-transpose (2-byte
    # granularity) can be used straight out of HBM.
    a_in = nc.dram_tensor("a", [BPC, M, K // 2], mybir.dt.uint16, kind="ExternalInput")
    b_in = nc.dram_tensor("b", [BPC, N, K // 2], mybir.dt.uint16, kind="ExternalInput")
    out = nc.dram_tensor("out", [BPC, M, N], mybir.dt.int32, kind="ExternalOutput")

    n_mt = M // M_TILE
    n_nt = N // N_TILE
    n_kt = 2 * KB

    with tile.TileContext(nc) as tc:
        with (
            tc.tile_pool(name="stage", bufs=1) as stage_pool,
            tc.tile_pool(name="conv", bufs=2) as conv_pool,
            tc.tile_pool(name="psum", bufs=8, space="PSUM") as psum_pool,
            tc.tile_pool(name="outbuf", bufs=8) as out_pool,
            tc.tile_pool(name="warm", bufs=1) as warm_pool,
        ):
            # PE warmup: dummy matmuls with NO deps (uninitialized SBUF
            # reads are fine; the PSUM result is discarded) ramp the HAM
            # clock gate while the first transposes land.
            wsrc = warm_pool.tile([128, N_TILE], mybir.dt.bfloat16, name="wsrc")
            nc.vector.memset(wsrc[:, :8], 0.0)
            wps = psum_pool.tile([128, N_TILE], mybir.dt.float32, name="wps", tag="ps")
            for _ in range(WARMUP_MMS):
                nc.tensor.matmul(wps[:], wsrc[:, :128], wsrc[:], start=True, stop=True)

            # ---- all transposes up front on SYNC. Batch 0: k-block 0 in
            # halves ordered B-h0, A-h0, B-h1 (the first matmuls need
            # B-h0 + A-h0), kb1-3 full tiles, A-h1 last. ----
            def stage_small(src, kb, name, m0, m1):
                t = stage_pool.tile(
                    [128, m1 - m0],
                    mybir.dt.uint16,
                    name=f"{name}0_{kb}_{m0}",
                    tag=f"{name}0_{kb}_{m0}",
                )
                nc.sync.dma_start_transpose(t[:], src[0, m0:m1, kb * 128 : (kb + 1) * 128])
                # (int8 view, m0, m1, byte offset of m0 within the tile)
                return (t.bitcast(mybir.dt.int8), m0, m1, 0)

            def stage_small_b(src, bi, kb, name):
                t = stage_pool.tile(
                    [128, M],
                    mybir.dt.uint16,
                    name=f"{name}{bi}_{kb}",
                    tag=f"{name}{bi}_{kb}",
                )
                nc.sync.dma_start_transpose(
                    t[:], src[bi, :, kb * 128 : (kb + 1) * 128]
                )
                return (t.bitcast(mybir.dt.int8), 0, M, 0)

            a_sm = {0: []}
            b_sm = {0: []}
            b_sm[0].append(stage_small(b_in, 0, "bt", 0, M // 2))
            a_sm[0].append(stage_small(a_in, 0, "at", 0, M // 2))
            b_sm[0].append(stage_small(b_in, 0, "bt", M // 2, M))
            for kb in range(1, KB):
                a_sm[kb] = [stage_small(a_in, kb, "at", 0, M)]
                b_sm[kb] = [stage_small(b_in, kb, "bt", 0, M)]
            a_sm[0].append(stage_small(a_in, 0, "at", M // 2, M))

            # Batches 1-3: batch 1 + batch 2's k-block 0 up front; the
            # rest thread through the store stream (each sync store pops
            # one pending transpose).
            a_smb = {}
            b_smb = {}

            def stage_one(kind, bi, kb):
                if kind == "A":
                    a_smb.setdefault(bi, {})[kb] = [stage_small_b(a_in, bi, kb, "at")]
                else:
                    b_smb.setdefault(bi, {})[kb] = [stage_small_b(b_in, bi, kb, "bt")]

            for kb in range(KB):
                stage_one("A", 1, kb)
                stage_one("B", 1, kb)
            stage_one("A", 2, 0)
            stage_one("B", 2, 0)
            # Spread the threaded transposes so the sync queue never
            # carries stores + transposes above ~190GB/s in any window:
            # batch 2's tail k-blocks thread through batch 0's stores,
            # batch 3's kb0/kb1 through batch 1's, kb2/kb3 through batch
            # 2's (its data is consumed from ~94us; issued ~70-80us).
            pending_by_batch = {
                0: [(k, 2, kb) for kb in (1, 2, 3) for k in ("A", "B")],
                1: [(k, 3, kb) for kb in (0, 1) for k in ("A", "B")],
                2: [(k, 3, kb) for kb in (2, 3) for k in ("A", "B")],
            }
            pending_T = []

            for bi in range(BPC):
                pending_T = pending_by_batch.get(bi, [])
                # ---- deinterleave + int8 -> bf16 (DVE). lhs_ap[kt][mt]
                # and rhs_ap[kt][nt] index into whichever tile holds that
                # m/n range. ----
                lhs_ap = [[None] * n_mt for _ in range(n_kt)]
                rhs_ap = [[None] * n_nt for _ in range(n_kt)]
                deferred_a = []  # batch 0's (kb, A-h1 seg): deints go last
                for kb in range(KB):
                    if bi == 0:
                        # Batch 0 deints in HALVES (sliced from the full
                        # staged tiles for kb>=1): mt group 0 never reads
                        # A's upper half (deints deferred past kb3), and
                        # B's h0 unlocks each kt's nt0 matmuls sooner.
                        if kb == 0:
                            a_segs = a_sm[kb]
                            b_segs = b_sm[kb]
                        else:
                            at8 = a_sm[kb][0][0]
                            bt8 = b_sm[kb][0][0]
                            a_segs = [(at8, 0, M // 2, 0), (at8, M // 2, M, M)]
                            b_segs = [(bt8, 0, M // 2, 0), (bt8, M // 2, M, M)]
                        deferred_a += [(kb, s) for s in a_segs if s[1] != 0]
                        a_segs = [s for s in a_segs if s[1] == 0]
                    else:
                        a_segs = a_smb[bi][kb]
                        b_segs = b_smb[bi][kb]
                    for par in range(2):
                        kt = kb * 2 + par
                        # B deints first: each kt's first matmuls gate on
                        # B-h0 + A, and B's transpose is issued first.
                        for st8, m0, m1, base in b_segs:
                            bbf = conv_pool.tile(
                                [128, m1 - m0],
                                mybir.dt.bfloat16,
                                name=f"bbf_{bi}_{kt}_{m0}",
                                tag=f"bbf{kt}_{m0}",
                            )
                            # All deints on DVE: ACT's ACTIVATE-copy runs
                            # int8 deints ~1.65x slower.
                            nc.vector.tensor_copy(
                                bbf[:],
                                st8[:, base + par : base + 2 * (m1 - m0) : 2],
                            )
                            for nt in range(m0 // N_TILE, (m1 + N_TILE - 1) // N_TILE):
                                o = nt * N_TILE - m0
                                rhs_ap[kt][nt] = bbf[:, o : o + N_TILE]
                        for st8, m0, m1, base in a_segs:
                            abf = conv_pool.tile(
                                [128, m1 - m0],
                                mybir.dt.bfloat16,
                                name=f"abf_{bi}_{kt}_{m0}",
                                tag=f"abf{kt}_{m0}",
                            )
                            nc.vector.tensor_copy(
                                abf[:],
                                st8[:, base + par : base + 2 * (m1 - m0) : 2],
                            )
                            for mt in range(m0 // M_TILE, m1 // M_TILE):
                                o = mt * M_TILE - m0
                                lhs_ap[kt][mt] = abf[:, o : o + M_TILE]

                # Batch 0's A-h1 deints last on the DVE FIFO: that data
                # (lhs for mt 4-7) is only needed at mt group 1 (~14us
                # after the ramp starts), and its transposes issue late.
                for kb, (st8, m0, m1, base) in deferred_a:
                    for par in range(2):
                        kt = kb * 2 + par
                        abf = conv_pool.tile(
                            [128, m1 - m0],
                            mybir.dt.bfloat16,
                            name=f"abf_{bi}_{kt}_{m0}",
                            tag=f"abf{kt}_{m0}",
                        )
                        nc.vector.tensor_copy(
                            abf[:],
                            st8[:, base + par : base + 2 * (m1 - m0) : 2],
                        )
                        for mt in range(m0 // M_TILE, m1 // M_TILE):
                            o = mt * M_TILE - m0
                            lhs_ap[kt][mt] = abf[:, o : o + M_TILE]

                # ---- GEMM, accumulating in PSUM over kt. After each mt
                # row's two PSUM-freeing copies (ACT), the row's 512KB
                # store issues; batches 0/1 on SYNC (threading pending
                # transposes - issuing stores from SCALAR while
                # transposes are still pending on sync serializes both
                # DMA streams: measured +12us from 3-7us PE stalls at
                # both batch boundaries). Batches 2/3 split: even mt
                # rows SYNC / odd SCALAR (transpose stream fully
                # drained by then, so the parallel queues are safe and
                # the tail drains 2x faster). ----
                def emit_store(mt, ot):
                    if bi >= 2 and mt % 2 == 1 and not pending_T:
                        nc.scalar.dma_start(
                            out[bi, mt * M_TILE : (mt + 1) * M_TILE, :], ot[:]
                        )
                    else:
                        nc.sync.dma_start(
                            out[bi, mt * M_TILE : (mt + 1) * M_TILE, :], ot[:]
                        )
                        if pending_T:
                            stage_one(*pending_T.pop(0))

                if bi == 0:
                    # Batch 0 is rate-limited by the transpose + deint
                    # stream: iterate kt-outer over groups of 4 mt blocks
                    # (8 PSUM banks) so each arriving k-tile feeds 1.7us
                    # of real PE work. nt-outer inside each kt so the
                    # first 4 matmuls only need B-half0.
                    for g in range(n_mt // 4):
                        mts = range(4 * g, 4 * g + 4)
                        ps = {
                            (mt, nt): psum_pool.tile(
                                [128, N_TILE],
                                mybir.dt.float32,
                                name=f"ps_{bi}_{mt}_{nt}",
                                tag="ps",
                            )
                            for mt in mts
                            for nt in range(n_nt)
                        }
                        for kt in range(n_kt):
                            for nt in range(n_nt):
                                for mt in mts:
                                    nc.tensor.matmul(
                                        ps[(mt, nt)][:],
                                        lhs_ap[kt][mt],
                                        rhs_ap[kt][nt],
                                        start=(kt == 0),
                                        stop=(kt == n_kt - 1),
                                    )
                        for mt in mts:
                            ot = out_pool.tile(
                                [128, N], mybir.dt.int32, name=f"ot_{bi}_{mt}", tag="ot"
                            )
                            for nt in range(n_nt):
                                nc.scalar.copy(
                                    ot[:, nt * N_TILE : (nt + 1) * N_TILE],
                                    ps[(mt, nt)][:],
                                )
                            emit_store(mt, ot)
                elif bi == BPC - 1:
                    # Last batch: mt-outer, but the FINAL row runs
                    # nt-outer so nt0's copy+store overlap nt1's matmuls
                    # and the kernel tail waits only on the last 128KB.
                    for mt in range(n_mt - 1):
                        ps = [
                            psum_pool.tile(
                                [128, N_TILE],
                                mybir.dt.float32,
                                name=f"ps_{bi}_{mt}_{nt}",
                                tag="ps",
                            )
                            for nt in range(n_nt)
                        ]
                        for kt in range(n_kt):
                            for nt in range(n_nt):
                                nc.tensor.matmul(
                                    ps[nt][:],
                                    lhs_ap[kt][mt],
                                    rhs_ap[kt][nt],
                                    start=(kt == 0),
                                    stop=(kt == n_kt - 1),
                                )
                        ot = out_pool.tile(
                            [128, N], mybir.dt.int32, name=f"ot_{bi}_{mt}", tag="ot"
                        )
                        for nt in range(n_nt):
                            nc.scalar.copy(
                                ot[:, nt * N_TILE : (nt + 1) * N_TILE], ps[nt][:]
                            )
                        emit_store(mt, ot)

                    mt = n_mt - 1
                    ps = [
                        psum_pool.tile(
                            [128, N_TILE],
                            mybir.dt.float32,
                            name=f"ps_{bi}_{mt}_{nt}",
                            tag="ps",
                        )
                        for nt in range(n_nt)
                    ]
                    # nt0 chain first, then nt1: nt0's copy + 256KB store
                    # run during nt1's 8 matmuls.
                    for nt in range(n_nt):
                        for kt in range(n_kt):
                            nc.tensor.matmul(
                                ps[nt][:],
                                lhs_ap[kt][mt],
                                rhs_ap[kt][nt],
                                start=(kt == 0),
                                stop=(kt == n_kt - 1),
                            )
                        if nt == 0:
                            o0 = out_pool.tile(
                                [128, N_TILE],
                                mybir.dt.int32,
                                name="ot_l0",
                                tag="otl0",
                                bufs=1,
                            )
                            nc.scalar.copy(o0[:], ps[0][:])
                            nc.scalar.dma_start(
                                out[bi, mt * M_TILE : (mt + 1) * M_TILE, :N_TILE],
                                o0[:],
                            )
                    # nt1: parallel ACT+DVE half copies, two 128KB stores
                    # on both queues; the tail waits only on the last one.
                    o1 = out_pool.tile(
                        [128, N_TILE],
                        mybir.dt.int32,
                        name="ot_l1",
                        tag="otl1",
                        bufs=1,
                    )
                    h = N_TILE // 2
                    nc.scalar.copy(o1[:, :h], ps[1][:, :h])
                    nc.vector.tensor_copy(o1[:, h:], ps[1][:, h:])
                    nc.scalar.dma_start(
                        out[bi, mt * M_TILE : (mt + 1) * M_TILE, N_TILE : N_TILE + h],
                        o1[:, :h],
                    )
                    nc.sync.dma_start(
                        out[bi, mt * M_TILE : (mt + 1) * M_TILE, N_TILE + h :],
                        o1[:, h:],
                    )
                else:
                    # Steady-state batches: mt-outer so the PSUM-freeing
                    # copies and stores spread evenly.
                    for mt in range(n_mt):
                        ps = [
                            psum_pool.tile(
                                [128, N_TILE],
                                mybir.dt.float32,
                                name=f"ps_{bi}_{mt}_{nt}",
                                tag="ps",
                            )
                            for nt in range(n_nt)
                        ]
                        for kt in range(n_kt):
                            for nt in range(n_nt):
                                nc.tensor.matmul(
                                    ps[nt][:],
                                    lhs_ap[kt][mt],
                                    rhs_ap[kt][nt],
                                    start=(kt == 0),
                                    stop=(kt == n_kt - 1),
                                )
                        ot = out_pool.tile(
                            [128, N], mybir.dt.int32, name=f"ot_{bi}_{mt}", tag="ot"
                        )
                        for nt in range(n_nt):
                            nc.scalar.copy(
                                ot[:, nt * N_TILE : (nt + 1) * N_TILE], ps[nt][:]
                            )
                        emit_store(mt, ot)
    nc.compile()
    return nc


def _get_nc():
    global _nc_cache
    if _nc_cache is None:
        _nc_cache = build_nc()
    return _nc_cache


def run(a: np.ndarray, b: np.ndarray, trace: bool = False):
    """Run on 8 cores. a/b: [32, 1024, 1024] int8. Returns (out, BassKernelResults)."""
    a = np.ascontiguousarray(a)
    b = np.ascontiguousarray(b)
    a16 = a.view(np.uint16).reshape(B, M, K // 2)
    b16 = b.view(np.uint16).reshape(B, N, K // 2)
    in_maps = [
        {
            "a": a16[c * BPC : (c + 1) * BPC],
            "b": b16[c * BPC : (c + 1) * BPC],
        }
        for c in range(N_CORES)
    ]
    res = run_bass_kernel_spmd(_get_nc(), in_maps, list(range(N_CORES)), trace=trace)
    out = np.concatenate([res.results[c]["out"] for c in range(N_CORES)], axis=0)
    return out, res


def kernel(a: np.ndarray, b: np.ndarray) -> np.ndarray:
    out, _ = run(np.asarray(a), np.asarray(b))
    return out


# revision 11
# speedup vs baseline: 1.1151x; 1.0297x over previous
"""Batched int8 GEMM (s8t x s8n -> s32t) on 8 TRN2 NeuronCores.

out[b, m, n] = sum_k a[b, m, k] * b[b, n, k]   (int32 accumulation)
a: [32, 1024, 1024] int8, b: [32, 1024, 1024] int8 -> out: [32, 1024, 1024] int32

Strategy:
  - Pure batch parallelism: 4 batches per core across 8 cores.
  - Both operands have K innermost, but the PE needs K on partitions.
    DMA-transpose works on 2-byte elements only, so we view the int8
    inputs as uint16 (pairs of adjacent K values) and DMA-transpose
    K-blocks of 256 K-values; each partition holds an even/odd K pair
    interleaved along the free dim. DVE deinterleaves (stride-2 int8
    reads) and converts int8 -> bf16: int8 is exact in bf16; products
    <= 2^14 and sums <= 2^24 are exact in fp32 PSUM accumulation, so
    the GEMM is bit-exact.
  - ALL transposes on the SYNC HWDGE queue: concurrent DMA-transposes
    on the sync+scalar queues race on the shared xbar and corrupt the
    staged tiles (measured). DMA_TRANSPOSE issue cost is ~1.3us FLAT
    regardless of size, so k-blocks transpose as FULL [1024,128]u16
    tiles (halves only where latency matters: batch 0 k-block 0, and
    A's upper half which is only needed at mt group 1).
  - Batch 0 issue order B-h0, A-h0, B-h1, then kb1-3 A+B fulls: the
    first matmul gates on B-h0+A-h0 (~1us earlier than A-first), and
    each arriving k-block feeds 1.7us of PE work (kt-outer over two
    groups of 4 mt blocks = 8 PSUM banks). Deints for B are emitted
    before A within each kt for the same reason. 9 dep-free dummy
    matmuls warm the HAM clock gate until the first real matmul.
  - Batch 1 + batch 2's first k-block transposes issue up front; the
    rest thread through batch 0/1's store stream on the sync FIFO
    (store, transpose, store, ... keeps every DMA's completion-
    semaphore-lane predecessor recent AND keeps stores flowing so the
    8-deep output-buffer ring never backs up into the ACT copies).
  - ACT copies PSUM fp32 -> SBUF int32 (exact: values are integers).
    Batches 0/1 stores issue from SYNC (interleaved with the pending
    transposes, as above). Batches 2/3 split: even mt rows on SYNC,
    odd on SCALAR - by then the transpose stream has fully drained,
    so the two store queues run in parallel and the tail drains 2x
    faster (single-queue stores previously backed up ~3us at the
    end).
  - The final mt row runs nt-outer (kt inner per nt) so nt0's copy +
    256KB store overlap nt1's matmuls; nt1 finishes with parallel
    ACT+DVE half copies and two 128KB stores on both queues, so the
    kernel tail waits only on the last 128KB.
"""

import numpy as np

import concourse.mybir as mybir
import concourse.tile as tile
from concourse import bacc
from concourse.bass_utils import run_bass_kernel_spmd

B, M, N, K = 32, 1024, 1024, 1024
N_CORES = 8
BPC = B // N_CORES  # batches per core
KB = K // 256  # k-blocks of 256 K-values (128 uint16 partitions)
N_TILE = 512
M_TILE = 128
WARMUP_MMS = 9

_nc_cache = None


def build_nc():
    nc = bacc.Bacc("TRN2")

    # int8 inputs viewed as uint16 so the xbar DMA-transpose (2-byte
    # granularity) can be used straight out of HBM.
    a_in = nc.dram_tensor("a", [BPC, M, K // 2], mybir.dt.uint16, kind="ExternalInput")
    b_in = nc.dram_tensor("b", [BPC, N, K // 2], mybir.dt.uint16, kind="ExternalInput")
    out = nc.dram_tensor("out", [BPC, M, N], mybir.dt.int32, kind="ExternalOutput")

    n_mt = M // M_TILE
    n_nt = N // N_TILE
    n_kt = 2 * KB

    with tile.TileContext(nc) as tc:
        with (
            tc.tile_pool(name="stage", bufs=1) as stage_pool,
            tc.tile_pool(name="conv", bufs=2) as conv_pool,
            tc.tile_pool(name="psum", bufs=8, space="PSUM") as psum_pool,
            tc.tile_pool(name="outbuf", bufs=8) as out_pool,
            tc.tile_pool(name="warm", bufs=1) as warm_pool,
        ):
            # PE warmup: dummy matmuls with NO deps (uninitialized SBUF
            # reads are fine; the PSUM result is discarded) ramp the HAM
            # clock gate while the first transposes land.
            wsrc = warm_pool.tile([128, N_TILE], mybir.dt.bfloat16, name="wsrc")
            nc.vector.memset(wsrc[:, :8], 0.0)
            wps = psum_pool.tile([128, N_TILE], mybir.dt.float32, name="wps", tag="ps")
            for _ in range(WARMUP_MMS):
                nc.tensor.matmul(wps[:], wsrc[:, :128], wsrc[:], start=True, stop=True)

            # ---- all transposes up front on SYNC. Batch 0: k-block 0 in
            # halves ordered B-h0, A-h0, B-h1 (the first matmuls need
            # B-h0 + A-h0), kb1-3 full tiles, A-h1 last. ----
            def stage_small(src, kb, name, m0, m1):
                t = stage_pool.tile(
                    [128, m1 - m0],
                    mybir.dt.uint16,
                    name=f"{name}0_{kb}_{m0}",
                    tag=f"{name}0_{kb}_{m0}",
                )
                nc.sync.dma_start_transpose(t[:], src[0, m0:m1, kb * 128 : (kb + 1) * 128])
                # (int8 view, m0, m1, byte offset of m0 within the tile)
                return (t.bitcast(mybir.dt.int8), m0, m1, 0)

            def stage_small_b(src, bi, kb, name):
                t = stage_pool.tile(
                    [128, M],
                    mybir.dt.uint16,
                    name=f"{name}{bi}_{kb}",
                    tag=f"{name}{bi}_{kb}",
                )
                nc.sync.dma_start_transpose(
                    t[:], src[bi, :, kb * 128 : (kb + 1) * 128]
                )
                return (t.bitcast(mybir.dt.int8), 0, M, 0)

            a_sm = {0: []}
            b_sm = {0: []}
            b_sm[0].append(stage_small(b_in, 0, "bt", 0, M // 2))
            a_sm[0].append(stage_small(a_in, 0, "at", 0, M // 2))
            b_sm[0].append(stage_small(b_in, 0, "bt", M // 2, M))
            for kb in range(1, KB):
                a_sm[kb] = [stage_small(a_in, kb, "at", 0, M)]
                b_sm[kb] = [stage_small(b_in, kb, "bt", 0, M)]
            a_sm[0].append(stage_small(a_in, 0, "at", M // 2, M))

            # Batches 1-3: batch 1 + batch 2's k-block 0 up front; the
            # rest thread through the store stream (each sync store pops
            # one pending transpose).
            a_smb = {}
            b_smb = {}

            def stage_one(kind, bi, kb):
                if kind == "A":
                    a_smb.setdefault(bi, {})[kb] = [stage_small_b(a_in, bi, kb, "at")]
                else:
                    b_smb.setdefault(bi, {})[kb] = [stage_small_b(b_in, bi, kb, "bt")]

            for kb in range(KB):
                stage_one("A", 1, kb)
                stage_one("B", 1, kb)
            stage_one("A", 2, 0)
            stage_one("B", 2, 0)
            pending_by_batch = {
                0: [(k, 2, kb) for kb in (1, 2, 3) for k in ("A", "B")],
                1: [(k, 3, kb) for kb in range(KB) for k in ("A", "B")],
            }
            pending_T = []

            for bi in range(BPC):
                pending_T = pending_by_batch.get(bi, [])
                # ---- deinterleave + int8 -> bf16 (DVE). lhs_ap[kt][mt]
                # and rhs_ap[kt][nt] index into whichever tile holds that
                # m/n range. ----
                lhs_ap = [[None] * n_mt for _ in range(n_kt)]
                rhs_ap = [[None] * n_nt for _ in range(n_kt)]
                deferred_a = []  # batch 0's (kb, A-h1 seg): deints go last
                for kb in range(KB):
                    if bi == 0:
                        # Batch 0 deints in HALVES (sliced from the full
                        # staged tiles for kb>=1): mt group 0 never reads
                        # A's upper half (deints deferred past kb3), and
                        # B's h0 unlocks each kt's nt0 matmuls sooner.
                        if kb == 0:
                            a_segs = a_sm[kb]
                            b_segs = b_sm[kb]
                        else:
                            at8 = a_sm[kb][0][0]
                            bt8 = b_sm[kb][0][0]
                            a_segs = [(at8, 0, M // 2, 0), (at8, M // 2, M, M)]
                            b_segs = [(bt8, 0, M // 2, 0), (bt8, M // 2, M, M)]
                        deferred_a += [(kb, s) for s in a_segs if s[1] != 0]
                        a_segs = [s for s in a_segs if s[1] == 0]
                    else:
                        a_segs = a_smb[bi][kb]
                        b_segs = b_smb[bi][kb]
                    for par in range(2):
                        kt = kb * 2 + par
                        # B deints first: each kt's first matmuls gate on
                        # B-h0 + A, and B's transpose is issued first.
                        for st8, m0, m1, base in b_segs:
                            bbf = conv_pool.tile(
                                [128, m1 - m0],
                                mybir.dt.bfloat16,
                                name=f"bbf_{bi}_{kt}_{m0}",
                                tag=f"bbf{kt}_{m0}",
                            )
                            # All deints on DVE: ACT's ACTIVATE-copy runs
                            # int8 deints ~1.65x slower.
                            nc.vector.tensor_copy(
                                bbf[:],
                                st8[:, base + par : base + 2 * (m1 - m0) : 2],
                            )
                            for nt in range(m0 // N_TILE, (m1 + N_TILE - 1) // N_TILE):
                                o = nt * N_TILE - m0
                                rhs_ap[kt][nt] = bbf[:, o : o + N_TILE]
                        for st8, m0, m1, base in a_segs:
                            abf = conv_pool.tile(
                                [128, m1 - m0],
                                mybir.dt.bfloat16,
                                name=f"abf_{bi}_{kt}_{m0}",
                                tag=f"abf{kt}_{m0}",
                            )
                            nc.vector.tensor_copy(
                                abf[:],
                                st8[:, base + par : base + 2 * (m1 - m0) : 2],
                            )
                            for mt in range(m0 // M_TILE, m1 // M_TILE):
                                o = mt * M_TILE - m0
                                lhs_ap[kt][mt] = abf[:, o : o + M_TILE]

                # Batch 0's A-h1 deints last on the DVE FIFO: that data
                # (lhs for mt 4-7) is only needed at mt group 1 (~14us
                # after the ramp starts), and its transposes issue late.
                for kb, (st8, m0, m1, base) in deferred_a:
                    for par in range(2):
                        kt = kb * 2 + par
                        abf = conv_pool.tile(
                            [128, m1 - m0],
                            mybir.dt.bfloat16,
                            name=f"abf_{bi}_{kt}_{m0}",
                            tag=f"abf{kt}_{m0}",
                        )
                        nc.vector.tensor_copy(
                            abf[:],
                            st8[:, base + par : base + 2 * (m1 - m0) : 2],
                        )
                        for mt in range(m0 // M_TILE, m1 // M_TILE):
                            o = mt * M_TILE - m0
                            lhs_ap[kt][mt] = abf[:, o : o + M_TILE]

                # ---- GEMM, accumulating in PSUM over kt. After each mt
                # row's two PSUM-freeing copies (ACT), the row's 512KB
                # store issues; batches 0/1 on SYNC (threading pending
                # transposes - issuing stores from SCALAR while
                # transposes are still pending on sync serializes both
                # DMA streams: measured +12us from 3-7us PE stalls at
                # both batch boundaries). Batches 2/3 split: even mt
                # rows SYNC / odd SCALAR (transpose stream fully
                # drained by then, so the parallel queues are safe and
                # the tail drains 2x faster). ----
                def emit_store(mt, ot):
                    if bi >= 2 and mt % 2 == 1:
                        nc.scalar.dma_start(
                            out[bi, mt * M_TILE : (mt + 1) * M_TILE, :], ot[:]
                        )
                    else:
                        nc.sync.dma_start(
                            out[bi, mt * M_TILE : (mt + 1) * M_TILE, :], ot[:]
                        )
                        if pending_T:
                            stage_one(*pending_T.pop(0))

                if bi == 0:
                    # Batch 0 is rate-limited by the transpose + deint
                    # stream: iterate kt-outer over groups of 4 mt blocks
                    # (8 PSUM banks) so each arriving k-tile feeds 1.7us
                    # of real PE work. nt-outer inside each kt so the
                    # first 4 matmuls only need B-half0.
                    for g in range(n_mt // 4):
                        mts = range(4 * g, 4 * g + 4)
                        ps = {
                            (mt, nt): psum_pool.tile(
                                [128, N_TILE],
                                mybir.dt.float32,
                                name=f"ps_{bi}_{mt}_{nt}",
                                tag="ps",
                            )
                            for mt in mts
                            for nt in range(n_nt)
                        }
                        for kt in range(n_kt):
                            for nt in range(n_nt):
                                for mt in mts:
                                    nc.tensor.matmul(
                                        ps[(mt, nt)][:],
                                        lhs_ap[kt][mt],
                                        rhs_ap[kt][nt],
                                        start=(kt == 0),
                                        stop=(kt == n_kt - 1),
                                    )
                        for mt in mts:
                            ot = out_pool.tile(
                                [128, N], mybir.dt.int32, name=f"ot_{bi}_{mt}", tag="ot"
                            )
                            for nt in range(n_nt):
                                nc.scalar.copy(
                                    ot[:, nt * N_TILE : (nt + 1) * N_TILE],
                                    ps[(mt, nt)][:],
                                )
                            emit_store(mt, ot)
                elif bi == BPC - 1:
                    # Last batch: mt-outer, but the FINAL row runs
                    # nt-outer so nt0's copy+store overlap nt1's matmuls
                    # and the kernel tail waits only on the last 128KB.
                    for mt in range(n_mt - 1):
                        ps = [
                            psum_pool.tile(
                                [128, N_TILE],
                                mybir.dt.float32,
                                name=f"ps_{bi}_{mt}_{nt}",
                                tag="ps",
                            )
                            for nt in range(n_nt)
                        ]
                        for kt in range(n_kt):
                            for nt in range(n_nt):
                                nc.tensor.matmul(
                                    ps[nt][:],
                                    lhs_ap[kt][mt],
                                    rhs_ap[kt][nt],
                                    start=(kt == 0),
                                    stop=(kt == n_kt - 1),
                                )
                        ot = out_pool.tile(
                            [128, N], mybir.dt.int32, name=f"ot_{bi}_{mt}", tag="ot"
                        )
                        for nt in range(n_nt):
                            nc.scalar.copy(
                                ot[:, nt * N_TILE : (nt + 1) * N_TILE], ps[nt][:]
                            )
                        emit_store(mt, ot)

                    mt = n_mt - 1
                    ps = [
                        psum_pool.tile(
                            [128, N_TILE],
                            mybir.dt.float32,
                            name=f"ps_{bi}_{mt}_{nt}",
                            tag="ps",
                        )
                        for nt in range(n_nt)
                    ]
                    # nt0 chain first, then nt1: nt0's copy + 256KB store
                    # run during nt1's 8 matmuls.
                    for nt in range(n_nt):
                        for kt in range(n_kt):
                            nc.tensor.matmul(
                                ps[nt][:],
                                lhs_ap[kt][mt],
                                rhs_ap[kt][nt],
                                start=(kt == 0),
                                stop=(kt == n_kt - 1),
                            )
                        if nt == 0:
                            o0 = out_pool.tile(
                                [128, N_TILE],
                                mybir.dt.int32,
                                name="ot_l0",
                                tag="otl0",
                                bufs=1,
                            )
                            nc.scalar.copy(o0[:], ps[0][:])
                            nc.scalar.dma_start(
                                out[bi, mt * M_TILE : (mt + 1) * M_TILE, :N_TILE],
                                o0[:],
                            )
                    # nt1: parallel ACT+DVE half copies, two 128KB stores
                    # on both queues; the tail waits only on the last one.
                    o1 = out_pool.tile(
                        [128, N_TILE],
                        mybir.dt.int32,
                        name="ot_l1",
                        tag="otl1",
                        bufs=1,
                    )
                    h = N_TILE // 2
                    nc.scalar.copy(o1[:, :h], ps[1][:, :h])
                    nc.vector.tensor_copy(o1[:, h:], ps[1][:, h:])
                    nc.scalar.dma_start(
                        out[bi, mt * M_TILE : (mt + 1) * M_TILE, N_TILE : N_TILE + h],
                        o1[:, :h],
                    )
                    nc.sync.dma_start(
                        out[bi, mt * M_TILE : (mt + 1) * M_TILE, N_TILE + h :],
                        o1[:, h:],
                    )
                else:
                    # Steady-state batches: mt-outer so the PSUM-freeing
                    # copies and stores spread evenly.
                    for mt in range(n_mt):
                        ps = [
                            psum_pool.tile(
                                [128, N_TILE],
                                mybir.dt.float32,
                                name=f"ps_{bi}_{mt}_{nt}",
                                tag="ps",
                            )
                            for nt in range(n_nt)
                        ]
                        for kt in range(n_kt):
                            for nt in range(n_nt):
                                nc.tensor.matmul(
                                    ps[nt][:],
                                    lhs_ap[kt][mt],
                                    rhs_ap[kt][nt],
                                    start=(kt == 0),
                                    stop=(kt == n_kt - 1),
                                )
                        ot = out_pool.tile(
                            [128, N], mybir.dt.int32, name=f"ot_{bi}_{mt}", tag="ot"
                        )
                        for nt in range(n_nt):
                            nc.scalar.copy(
                                ot[:, nt * N_TILE : (nt + 1) * N_TILE], ps[nt][:]
                            )
                        emit_store(mt, ot)
    nc.compile()
    return nc


def _get_nc():
    global _nc_cache
    if _nc_cache is None:
        _nc_cache = build_nc()
    return _nc_cache


def run(a: np.ndarray, b: np.ndarray, trace: bool = False):
    """Run on 8 cores. a/b: [32, 1024, 1024] int8. Returns (out, BassKernelResults)."""
    a = np.ascontiguousarray(a)
    b = np.ascontiguousarray(b)
    a16 = a.view(np.uint16).reshape(B, M, K // 2)
    b16 = b.view(np.uint16).reshape(B, N, K // 2)
    in_maps = [
        {
            "a": a16[c * BPC : (c + 1) * BPC],
            "b": b16[c * BPC : (c + 1) * BPC],
        }
        for c in range(N_CORES)
    ]
    res = run_bass_kernel_spmd(_get_nc(), in_maps, list(range(N_CORES)), trace=trace)
    out = np.concatenate([res.results[c]["out"] for c in range(N_CORES)], axis=0)
    return out, res


def kernel(a: np.ndarray, b: np.ndarray) -> np.ndarray:
    out, _ = run(np.asarray(a), np.asarray(b))
    return out
